# revision 8
# baseline (speedup 1.0000x reference)
"""Trainium2 Bass kernel for nn_AttentionLayer (B=16, T=2048, D=256), 8 cores.

Math (per batch b):
    h  = input[:, :256] + input[:, 256:512]            # [T, D]
    aw = relu(h @ W.T + b)                             # [T, D]
    m  = tanh(h)
    S  = m @ aw.T                                      # [T, T]
    P  = softmax(S, axis=-1)
    out = h.T + h.T @ P                                # [D, T]

Sharding: data-parallel over batch. 16 batches -> 2 per NeuronCore.

Kernel structure per batch (all bf16 on the TensorEngine, f32 PSUM accum):
    S1 : DMA input row-blocks [128, 512], h_td = half0 + half1 (bf16, t on
         partitions).
    S1b: transpose h_td via identity-matmul -> hT (d on partitions);
         mT = tanh(hT) on ScalarE straight from PSUM.
    S2 : awT = relu(W.T-matmul + bias) -> [d_half][128, T] bf16.
    S3 : per t-block: S = mT.T @ awT (PSUM), E = exp(S - 45) on ScalarE with
         fused row-sum accumulation; g = h_td / rowsum (DVE).
         The constant -45 shift is mathematically exact for softmax (cancels
         in the normalization) and keeps exp() in f32/bf16 range.
    S4 : out[d_half, s_chunk] = sum_t g[t, d] * E[t, s] accumulated over the
         16 t-blocks in PSUM; residual h.T added during the PSUM->SBUF copy.
"""

import numpy as np

import concourse.bass as bass
import concourse.mybir as mybir
import concourse.tile as tile
from concourse import bacc
from concourse.bass_utils import run_bass_kernel_spmd
from concourse.masks import make_identity

N_CORES = 8
B_FULL, T_FULL, D2 = 16, 2048, 512
D = 256
EXP_SHIFT = -45.0  # exact for softmax; bounds exp() inputs


def build_kernel(nc, b_loc: int, t: int, d: int):
    """Emit the Tile program. t = seq len, d = feature dim (256)."""
    f32 = mybir.dt.float32
    bf16 = mybir.dt.bfloat16
    P = 128
    ntb = t // P          # t-blocks per batch
    d_halves = d // P     # 2
    SC = min(1024, t)     # free-dim chunk for psum tiles / ACT ops
    MMN = min(512, SC)    # max moving-operand width per matmul
    nsc = t // SC         # s-chunks per row

    inp = nc.dram_tensor("input_feature", [b_loc, t, 2 * d], f32,
                         kind="ExternalInput").ap()
    W = nc.dram_tensor("W", [d, d], f32, kind="ExternalInput").ap()
    bias = nc.dram_tensor("b", [d], f32, kind="ExternalInput").ap()
    out = nc.dram_tensor("out", [b_loc, d, t], f32,
                         kind="ExternalOutput").ap()

    with tile.TileContext(nc) as tc:
        with (
            tc.tile_pool(name="const", bufs=1) as const,
            tc.tile_pool(name="ps", bufs=4, space="PSUM") as ps,
            tc.tile_pool(name="inp_p", bufs=4) as inp_p,
            tc.tile_pool(name="h_p", bufs=2 * ntb) as h_p,
            tc.tile_pool(name="g_p", bufs=2 * ntb) as g_p,
            tc.tile_pool(name="hT_p", bufs=2 * d_halves) as hT_p,
            tc.tile_pool(name="mT_p", bufs=2 * d_halves) as mT_p,
            tc.tile_pool(name="awT_p", bufs=2 * d_halves) as awT_p,
            tc.tile_pool(name="E_p", bufs=min(2 * ntb, ntb + 4)) as E_p,
            tc.tile_pool(name="z_p", bufs=8) as z_p,
            tc.tile_pool(name="out_p", bufs=3) as out_p,
        ):
            # ---- setup: identity, W^T (bf16), bias ----
            ident = const.tile([P, P], bf16)
            make_identity(nc, ident[:])

            b_sb = const.tile([P, d_halves], f32)
            nc.sync.dma_start(out=b_sb[:], in_=bias.rearrange("(h p) -> p h", p=P))

            shift = const.tile([P, 1], f32)
            nc.vector.memset(shift[:], EXP_SHIFT)

            w_stage = []
            for k in range(d_halves):
                wf = const.tile([P, d], f32, tag=f"w_f32_{k}")
                nc.sync.dma_start(out=wf[:], in_=W[k * P:(k + 1) * P, :])
                wb = const.tile([P, d], bf16, tag=f"w_bf16_{k}")
                nc.vector.tensor_copy(wb[:], wf[:])
                w_stage.append(wb)

            # WT[dh] holds W.T rows d in [dh*128, ...), cols e = 0..d
            WT = []
            for dh in range(d_halves):
                ps_w = ps.tile([P, SC], f32, tag="ps")
                for k in range(d_halves):
                    nc.tensor.matmul(ps_w[:, k * P:(k + 1) * P],
                                     w_stage[k][:, dh * P:(dh + 1) * P],
                                     ident[:], start=True, stop=True)
                wt = const.tile([P, d], bf16, tag=f"wt_{dh}")
                nc.vector.tensor_copy(wt[:], ps_w[:, 0:d])
                WT.append(wt)

            for b in range(b_loc):
                # ---- S1: load input, h = h1 + h2 (bf16, t on partitions) ----
                h_td = []
                for tb in range(ntb):
                    it = inp_p.tile([P, 2 * d], f32, tag="in")
                    nc.sync.dma_start(out=it[:], in_=inp[b, tb * P:(tb + 1) * P, :])
                    ht = h_p.tile([P, d], bf16, tag="h")
                    nc.vector.tensor_add(ht[:], it[:, 0:d], it[:, d:2 * d])
                    h_td.append(ht)

                # ---- S1b: transpose -> hT (bf16), mT = tanh(hT) ----
                hT = [hT_p.tile([P, t], bf16, tag="hT", name=f"hT{_i}") for _i in range(d_halves)]
                mT = [mT_p.tile([P, t], bf16, tag="mT", name=f"mT{_i}") for _i in range(d_halves)]
                q_blocks = SC // P  # t-blocks per psum tile
                for dh in range(d_halves):
                    for q in range(ntb // q_blocks):
                        ps_t = ps.tile([P, SC], f32, tag="ps")
                        for j in range(q_blocks):
                            tb = q * q_blocks + j
                            nc.tensor.matmul(ps_t[:, j * P:(j + 1) * P],
                                             h_td[tb][:, dh * P:(dh + 1) * P],
                                             ident[:], start=True, stop=True)
                        sl = slice(q * SC, (q + 1) * SC)
                        nc.scalar.activation(mT[dh][:, sl], ps_t[:],
                                             mybir.ActivationFunctionType.Tanh)
                        nc.vector.tensor_copy(hT[dh][:, sl], ps_t[:])

                # ---- S2: awT = relu(W.T @ h.T + b) ----
                awT = [awT_p.tile([P, t], bf16, tag="awT", name=f"awT{_i}") for _i in range(d_halves)]
                for eh in range(d_halves):
                    for sc in range(nsc):
                        ps_aw = ps.tile([P, SC], f32, tag="ps")
                        sl = slice(sc * SC, (sc + 1) * SC)
                        for n0 in range(0, SC, MMN):
                            for k in range(d_halves):
                                nc.tensor.matmul(
                                    ps_aw[:, n0:n0 + MMN],
                                    WT[k][:, eh * P:(eh + 1) * P],
                                    hT[k][:, sc * SC + n0:sc * SC + n0 + MMN],
                                    start=(k == 0), stop=(k == d_halves - 1))
                        nc.vector.tensor_scalar(
                            out=awT[eh][:, sl], in0=ps_aw[:],
                            scalar1=b_sb[:, eh:eh + 1], scalar2=0.0,
                            op0=mybir.AluOpType.add, op1=mybir.AluOpType.max)

                # ---- S3: S = m @ aw.T, E = exp(S + shift), g = h / rowsum ----
                E = [E_p.tile([P, t], bf16, tag="E", name=f"E{_i}") for _i in range(ntb)]
                g = [g_p.tile([P, d], bf16, tag="g", name=f"g{_i}") for _i in range(ntb)]
                for tb in range(ntb):
                    zp = z_p.tile([P, nsc], f32, tag="zp")
                    tsl = slice(tb * P, (tb + 1) * P)
                    for sc in range(nsc):
                        ps_s = ps.tile([P, SC], f32, tag="ps")
                        sl = slice(sc * SC, (sc + 1) * SC)
                        for n0 in range(0, SC, MMN):
                            for k in range(d_halves):
                                nc.tensor.matmul(
                                    ps_s[:, n0:n0 + MMN], mT[k][:, tsl],
                                    awT[k][:, sc * SC + n0:sc * SC + n0 + MMN],
                                    start=(k == 0), stop=(k == d_halves - 1))
                        nc.scalar.activation(E[tb][:, sl], ps_s[:],
                                             mybir.ActivationFunctionType.Exp,
                                             bias=shift[:], scale=1.0,
                                             accum_out=zp[:, sc:sc + 1])
                    if nsc == 1:
                        zs = zp
                    else:
                        zs = z_p.tile([P, 1], f32, tag="zs")
                        if nsc == 2:
                            nc.vector.tensor_add(zs[:], zp[:, 0:1], zp[:, 1:2])
                        else:
                            nc.vector.reduce_sum(zs[:], zp[:],
                                                 axis=mybir.AxisListType.X)
                    rinv = z_p.tile([P, 1], f32, tag="rinv")
                    nc.vector.reciprocal(rinv[:], zs[:])
                    nc.vector.tensor_scalar_mul(g[tb][:], h_td[tb][:], rinv[:])

                # ---- S4: out[dh, sc] = sum_tb g.T @ E + h.T ----
                for dh in range(d_halves):
                    for sc in range(nsc):
                        ps_o = ps.tile([P, SC], f32, tag="ps")
                        sl = slice(sc * SC, (sc + 1) * SC)
                        for n0 in range(0, SC, MMN):
                            for tb in range(ntb):
                                nc.tensor.matmul(
                                    ps_o[:, n0:n0 + MMN],
                                    g[tb][:, dh * P:(dh + 1) * P],
                                    E[tb][:, sc * SC + n0:sc * SC + n0 + MMN],
                                    start=(tb == 0), stop=(tb == ntb - 1))
                        ot = out_p.tile([P, SC], f32, tag="out")
                        nc.vector.tensor_add(ot[:], ps_o[:], hT[dh][:, sl])
                        nc.sync.dma_start(
                            out=out[b, dh * P:(dh + 1) * P, sc * SC:(sc + 1) * SC],
                            in_=ot[:])
    return nc


_COMPILED = {}


def _get_compiled(b_loc: int, t: int, d: int):
    key = (b_loc, t, d)
    if key not in _COMPILED:
        nc = bacc.Bacc("TRN2", target_bir_lowering=False, debug=False,
                       num_devices=N_CORES)
        build_kernel(nc, b_loc, t, d)
        nc.compile()
        _COMPILED[key] = nc
    return _COMPILED[key]


def kernel(input_feature: np.ndarray, W: np.ndarray, b: np.ndarray,
           trace: bool = False, **extra_kwargs):
    input_feature = np.ascontiguousarray(input_feature, dtype=np.float32)
    W = np.ascontiguousarray(W, dtype=np.float32)
    b = np.ascontiguousarray(b, dtype=np.float32)

    b_full, t, d2 = input_feature.shape
    b_loc = b_full // N_CORES
    nc = _get_compiled(b_loc, t, d2 // 2)

    in_maps = [
        {"input_feature": input_feature[i * b_loc:(i + 1) * b_loc], "W": W, "b": b}
        for i in range(N_CORES)
    ]
    res = run_bass_kernel_spmd(nc, in_maps, core_ids=list(range(N_CORES)),
                               trace=trace, **extra_kwargs)
    out = np.concatenate([r["out"] for r in res.results], axis=0)
    if trace:
        kernel.last_result = res
    return out


# revision 9
# speedup vs baseline: 1.0854x; 1.0854x over previous
"""Trainium2 Bass kernel for nn_AttentionLayer (B=16, T=2048, D=256), 8 cores.

Math (per batch b):
    h  = input[:, :256] + input[:, 256:512]            # [T, D]
    aw = relu(h @ W.T + b)                             # [T, D]
    m  = tanh(h)
    S  = m @ aw.T                                      # [T, T]
    P  = softmax(S, axis=-1)
    out = h.T + h.T @ P                                # [D, T]

Sharding: data-parallel over batch. 16 batches -> 2 per NeuronCore.

Kernel structure per batch (all bf16 on the TensorEngine, f32 PSUM accum):
    S1 : DMA input row-blocks [128, 512], h_td = half0 + half1 (bf16, t on
         partitions).
    S1b: transpose h_td via identity-matmul -> hT (d on partitions);
         mT = tanh(hT) on ScalarE straight from PSUM.
    S2 : awT = relu(W.T-matmul + bias) -> [d_half][128, T] bf16.
    S3 : per t-block: S = mT.T @ awT into a full-row [128, T] PSUM tile,
         E = exp(S - 45) in ONE ScalarE op with fused row-sum accumulation;
         g = h_td / rowsum (DVE).  The constant -45 shift is mathematically
         exact for softmax (cancels in normalization) and bounds exp().
    S4 : out[d_half] = sum_t g[t, d] * E[t, s] accumulated over the 16
         t-blocks into a full-row PSUM tile; residual h.T added during the
         PSUM->SBUF copy.

PSUM: one pool of [128, T<=2048] f32 tiles (4 banks), bufs=2.  Large tiles
keep ScalarE ops big (amortize the ~350-cycle ACTIVATE overhead) and keep
TensorE matmul streams dense so the HAM clock-gate stays at 2.4 GHz.
"""

import numpy as np

import concourse.bass as bass
import concourse.mybir as mybir
import concourse.tile as tile
from concourse import bacc
from concourse.bass_utils import run_bass_kernel_spmd
from concourse.masks import make_identity

N_CORES = 8
EXP_SHIFT = -45.0  # exact for softmax; bounds exp() inputs


def build_kernel(nc, b_loc: int, t: int, d: int):
    """Emit the Tile program. t = seq len, d = feature dim (256)."""
    f32 = mybir.dt.float32
    bf16 = mybir.dt.bfloat16
    P = 128
    MMN = 512             # max moving-operand width per matmul
    ntb = t // P          # t-blocks per batch
    d_halves = d // P     # 2

    inp = nc.dram_tensor("input_feature", [b_loc, t, 2 * d], f32,
                         kind="ExternalInput").ap()
    W = nc.dram_tensor("W", [d, d], f32, kind="ExternalInput").ap()
    bias = nc.dram_tensor("b", [d], f32, kind="ExternalInput").ap()
    out = nc.dram_tensor("out", [b_loc, d, t], f32,
                         kind="ExternalOutput").ap()

    with tile.TileContext(nc) as tc:
        with (
            tc.tile_pool(name="const", bufs=1) as const,
            tc.tile_pool(name="ps", bufs=2, space="PSUM") as ps,
            tc.tile_pool(name="inp_p", bufs=4) as inp_p,
            tc.tile_pool(name="h_p", bufs=2 * ntb) as h_p,
            tc.tile_pool(name="g_p", bufs=2 * ntb) as g_p,
            tc.tile_pool(name="hT_p", bufs=2 * d_halves) as hT_p,
            tc.tile_pool(name="mT_p", bufs=2 * d_halves) as mT_p,
            tc.tile_pool(name="awT_p", bufs=2 * d_halves) as awT_p,
            tc.tile_pool(name="E_p", bufs=min(2 * ntb, ntb + 4)) as E_p,
            tc.tile_pool(name="z_p", bufs=8) as z_p,
            tc.tile_pool(name="out_p", bufs=3) as out_p,
        ):
            # ---- setup: identity, W^T (bf16), bias ----
            ident = const.tile([P, P], bf16)
            make_identity(nc, ident[:])

            b_sb = const.tile([P, d_halves], f32)
            nc.sync.dma_start(out=b_sb[:], in_=bias.rearrange("(h p) -> p h", p=P))

            shift = const.tile([P, 1], f32)
            nc.vector.memset(shift[:], EXP_SHIFT)

            w_stage = []
            for k in range(d_halves):
                wf = const.tile([P, d], f32, tag=f"w_f32_{k}")
                nc.sync.dma_start(out=wf[:], in_=W[k * P:(k + 1) * P, :])
                wb = const.tile([P, d], bf16, tag=f"w_bf16_{k}")
                nc.vector.tensor_copy(wb[:], wf[:])
                w_stage.append(wb)

            # WT[dh] holds W.T rows d in [dh*128, ...), cols e = 0..d
            WT = []
            for dh in range(d_halves):
                ps_w = ps.tile([P, t], f32, tag="ps", name=f"ps_w{dh}")
                for k in range(d_halves):
                    nc.tensor.matmul(ps_w[:, k * P:(k + 1) * P],
                                     w_stage[k][:, dh * P:(dh + 1) * P],
                                     ident[:], start=True, stop=True)
                wt = const.tile([P, d], bf16, tag=f"wt_{dh}")
                nc.vector.tensor_copy(wt[:], ps_w[:, 0:d])
                WT.append(wt)

            for b in range(b_loc):
                # ---- S1: load input, h = h1 + h2 (bf16, t on partitions) ----
                h_td = []
                for tb in range(ntb):
                    it = inp_p.tile([P, 2 * d], f32, tag="in")
                    nc.sync.dma_start(out=it[:], in_=inp[b, tb * P:(tb + 1) * P, :])
                    ht = h_p.tile([P, d], bf16, tag="h")
                    nc.vector.tensor_add(ht[:], it[:, 0:d], it[:, d:2 * d])
                    h_td.append(ht)

                # ---- S1b: transpose -> hT (bf16), mT = tanh(hT) ----
                hT = [hT_p.tile([P, t], bf16, tag="hT", name=f"hT{i}")
                      for i in range(d_halves)]
                mT = [mT_p.tile([P, t], bf16, tag="mT", name=f"mT{i}")
                      for i in range(d_halves)]
                for dh in range(d_halves):
                    ps_t = ps.tile([P, t], f32, tag="ps", name=f"ps_t{dh}")
                    for tb in range(ntb):
                        nc.tensor.matmul(ps_t[:, tb * P:(tb + 1) * P],
                                         h_td[tb][:, dh * P:(dh + 1) * P],
                                         ident[:], start=True, stop=True)
                    nc.scalar.activation(mT[dh][:], ps_t[:],
                                         mybir.ActivationFunctionType.Tanh)
                    nc.vector.tensor_copy(hT[dh][:], ps_t[:])

                # ---- S2: awT = relu(W.T @ h.T + b) ----
                awT = [awT_p.tile([P, t], bf16, tag="awT", name=f"awT{i}")
                       for i in range(d_halves)]
                for eh in range(d_halves):
                    ps_aw = ps.tile([P, t], f32, tag="ps", name=f"ps_aw{eh}")
                    for n0 in range(0, t, MMN):
                        for k in range(d_halves):
                            nc.tensor.matmul(
                                ps_aw[:, n0:n0 + MMN],
                                WT[k][:, eh * P:(eh + 1) * P],
                                hT[k][:, n0:n0 + MMN],
                                start=(k == 0), stop=(k == d_halves - 1))
                    nc.vector.tensor_scalar(
                        out=awT[eh][:], in0=ps_aw[:],
                        scalar1=b_sb[:, eh:eh + 1], scalar2=0.0,
                        op0=mybir.AluOpType.add, op1=mybir.AluOpType.max)

                # ---- S3: S = m @ aw.T, E = exp(S + shift), g = h / rowsum ----
                E = [E_p.tile([P, t], bf16, tag="E", name=f"E{i}")
                     for i in range(ntb)]
                g = [g_p.tile([P, d], bf16, tag="g", name=f"g{i}")
                     for i in range(ntb)]
                for tb in range(ntb):
                    tsl = slice(tb * P, (tb + 1) * P)
                    ps_s = ps.tile([P, t], f32, tag="ps", name=f"ps_s{tb}")
                    for n0 in range(0, t, MMN):
                        for k in range(d_halves):
                            nc.tensor.matmul(
                                ps_s[:, n0:n0 + MMN], mT[k][:, tsl],
                                awT[k][:, n0:n0 + MMN],
                                start=(k == 0), stop=(k == d_halves - 1))
                    zs = z_p.tile([P, 1], f32, tag="zs")
                    nc.scalar.activation(E[tb][:], ps_s[:],
                                         mybir.ActivationFunctionType.Exp,
                                         bias=shift[:], scale=1.0,
                                         accum_out=zs[:])
                    rinv = z_p.tile([P, 1], f32, tag="rinv")
                    nc.vector.reciprocal(rinv[:], zs[:])
                    nc.vector.tensor_scalar_mul(g[tb][:], h_td[tb][:], rinv[:])

                # ---- S4: out[dh] = sum_tb g.T @ E + h.T ----
                for dh in range(d_halves):
                    ps_o = ps.tile([P, t], f32, tag="ps", name=f"ps_o{dh}")
                    for n0 in range(0, t, MMN):
                        for tb in range(ntb):
                            nc.tensor.matmul(
                                ps_o[:, n0:n0 + MMN],
                                g[tb][:, dh * P:(dh + 1) * P],
                                E[tb][:, n0:n0 + MMN],
                                start=(tb == 0), stop=(tb == ntb - 1))
                    ot = out_p.tile([P, t], f32, tag="out")
                    nc.vector.tensor_add(ot[:], ps_o[:], hT[dh][:])
                    nc.sync.dma_start(out=out[b, dh * P:(dh + 1) * P, :], in_=ot[:])
    return nc


_COMPILED = {}


def _get_compiled(b_loc: int, t: int, d: int):
    key = (b_loc, t, d)
    if key not in _COMPILED:
        nc = bacc.Bacc("TRN2", target_bir_lowering=False, debug=False,
                       num_devices=N_CORES)
        build_kernel(nc, b_loc, t, d)
        nc.compile()
        _COMPILED[key] = nc
    return _COMPILED[key]


def kernel(input_feature: np.ndarray, W: np.ndarray, b: np.ndarray,
           trace: bool = False, **extra_kwargs):
    input_feature = np.ascontiguousarray(input_feature, dtype=np.float32)
    W = np.ascontiguousarray(W, dtype=np.float32)
    b = np.ascontiguousarray(b, dtype=np.float32)

    b_full, t, d2 = input_feature.shape
    b_loc = b_full // N_CORES
    nc = _get_compiled(b_loc, t, d2 // 2)

    in_maps = [
        {"input_feature": input_feature[i * b_loc:(i + 1) * b_loc], "W": W, "b": b}
        for i in range(N_CORES)
    ]
    res = run_bass_kernel_spmd(nc, in_maps, core_ids=list(range(N_CORES)),
                               trace=trace, **extra_kwargs)
    out = np.concatenate([r["out"] for r in res.results], axis=0)
    if trace:
        kernel.last_result = res
    return out
